# revision 15
# baseline (speedup 1.0000x reference)
"""Trainium2 Bass kernel for ClusteringMMD.

Per graph (batch-sharded 16+16 graphs onto each of 8 cores):
  - host pre-permutes the [512,512] adjacency to [128,4,512] so the
    device DMA is one fully contiguous 1MB transfer
  - ScalarE: one 2048-wide cast f32 -> fp8e4 (values are exactly 0/1)
  - TensorE: A^2 = A @ A via fp8 DoubleRow matmuls into PSUM (exact:
    0/1 products, fp32 accumulate); deg = ones^T @ A via two more
    DoubleRow matmuls (column-sum = row-sum for symmetric A)
  - VectorE: scalar_tensor_tensor fuses X = A^2 * A with accum_out =
    row-sum(X), yielding tri2 = diag(A^3) per node in one pass
  - DMA out tri2 [128,4] and deg [1,512] per graph
Host: bit-exact f32 replication of the reference's clustering-coefficient
binning (tri2/deg are exact small integers, so the device result is exact),
then the tiny [128,100] histogram MMD in f64.

The walrus build in this container rejects instructions carrying more than
one sync wait; _patch_compiler_wait_split() rewrites the BIR JSON right
before compilation, moving excess waits onto same-engine NoOps inserted
immediately before the over-subscribed instruction.
"""

import json
import numpy as np

B = 128
N = 512
BINS = 100
SIGMA = 1.0
N_CORES = 8
PER = B // N_CORES          # graphs per input tensor per core
GP = 2 * PER                # graphs per core (adj_1 shard + adj_2 shard)
P = 128
T = N // P                  # 4 row-blocks

MM_DTYPE = "fp8"            # "fp8" (DoubleRow) or "bf16"
WAIT_CAP = 1                # max sync waits this walrus accepts per inst

_NC_CACHE = {}


def _split_waits(bir_json, cap=WAIT_CAP):
    """Rewrite BIR JSON so no instruction carries more than `cap` sync
    waits; excess waits move to NoOps inserted just before it on the same
    engine (per-engine program order is list order within a block)."""
    m = json.loads(bir_json)
    ctr = 0
    for fn in m.get("functions", []):
        for blk in fn.get("blocks", []):
            out = []
            changed = False
            for ins in blk.get("instructions", []):
                si = ins.get("sync_info")
                waits = (si or {}).get("on_wait") or []
                if len(waits) > cap:
                    changed = True
                    for i in range(0, len(waits) - cap, cap):
                        ctr += 1
                        out.append(
                            {
                                "debug": ins.get("debug", 0),
                                "engine": ins["engine"],
                                "ins": [],
                                "name": f"WSPLIT-{ctr}",
                                "opcode": "NoOp",
                                "outs": [],
                                "text_hint": "wait_split",
                                "sync_info": {
                                    "on_wait": waits[i : i + cap],
                                    "on_update": [],
                                },
                            }
                        )
                    si["on_wait"] = waits[len(waits) - cap :]
                out.append(ins)
            if changed:
                blk["instructions"] = out
    return json.dumps(m).encode()


def _patch_compiler_wait_split():
    import concourse.bass_utils as bu
    import concourse.bass2jax as b2j

    if getattr(bu, "_wait_split_patched", False):
        return
    orig = bu.compile_bir_kernel

    def wrapped(bir_json, tmpdir, neff_name="file.neff"):
        return orig(_split_waits(bir_json), tmpdir, neff_name)

    bu.compile_bir_kernel = wrapped
    b2j.compile_bir_kernel = wrapped
    bu._wait_split_patched = True


def _patch_lean_tail():
    """Drop the second all-engine barrier of the Tile epilogue: semaphores
    are cleared after the first barrier (all engines quiesced), and nothing
    executes after the epilogue within this NEFF iteration."""
    from concourse.tile import TileContext
    from concourse.vector_clock import ScopedClock

    if getattr(TileContext, "_lean_tail", False):
        return

    def patched(self, tick_clock, wait_clock):
        nc = self.nc
        drain_inst = nc.sync.drain()
        wait_clock.add_sem_waits(
            drain_inst.ins, ScopedClock({None: tick_clock.global_clock})
        )
        nc.all_engine_barrier()
        assert self.sems is not None
        popped = nc._tile_sem_poison_stack.pop()
        assert popped is self._sem_poison
        nc.clear_and_free_semaphores(list(self.sems.allocated().values()))

    TileContext._drain_and_barrier = patched
    TileContext._lean_tail = True


def build_nc(gp=GP, mm_dtype=MM_DTYPE):
    import concourse.bass as bass
    import concourse.mybir as mybir
    from concourse.tile import TileContext
    from contextlib import ExitStack

    _patch_compiler_wait_split()
    _patch_lean_tail()
    dt = mybir.dt
    fp8 = mm_dtype == "fp8"
    cast_dt = dt.float8e4 if fp8 else dt.bfloat16

    nc = bass.Bass(
        "TRN2", target_bir_lowering=False, debug=False, num_devices=N_CORES
    )
    # input pre-permuted on host: a[g, p, t, n] = A_g[t*128 + p, n]
    a = nc.declare_dram_parameter("a", [gp, P, T, N], dt.float32, isOutput=False)
    # tri2 partition-major: ot[p, g*T + m] = tri2_g[m*128 + p]
    ot = nc.declare_dram_parameter("ot", [P, gp * T], dt.float32, isOutput=True)
    od = nc.declare_dram_parameter("od", [gp * N], dt.float32, isOutput=True)

    with TileContext(nc) as tc, ExitStack() as ctx:
        pconst = ctx.enter_context(tc.tile_pool(name="const", bufs=1))
        paf = ctx.enter_context(tc.tile_pool(name="af", bufs=8))
        pa8 = ctx.enter_context(tc.tile_pool(name="a8", bufs=4))
        px = ctx.enter_context(tc.tile_pool(name="xs", bufs=2))
        pps = ctx.enter_context(tc.tile_pool(name="ps", bufs=6, space="PSUM"))
        pdg = ctx.enter_context(tc.tile_pool(name="dg", bufs=2, space="PSUM"))

        # all-ones stationary operand for the deg column-sum matmuls;
        # [128, 2, 16] so the DoubleRow Ko-pair step is 16 bytes
        ones8 = pconst.tile([P, 2, 16], cast_dt)
        nc.vector.memset(ones8[:], 1.0)
        # whole-core result staging, one output DMA each at the end
        st_all = pconst.tile([P, gp * T], dt.float32)
        dg_all = pconst.tile([1, gp * N], dt.float32)

        for g in range(gp):
            af = paf.tile([P, T, N], dt.float32)
            nc.sync.dma_start(out=af[:], in_=a[g])
            a8 = pa8.tile([P, T, N], cast_dt)
            nc.scalar.activation(
                a8[:, :, :],
                af[:, :, :],
                mybir.ActivationFunctionType.Copy,
            )
            # deg = column-sum(A) (= row-sum, A symmetric) on the PE
            dg = pdg.tile([1, N], dt.float32)
            if fp8:
                for kk in range(T // 2):
                    nc.tensor.matmul(
                        dg[:],
                        ones8[:, :, 0:1],
                        a8[:, 2 * kk : 2 * kk + 2, :],
                        start=(kk == 0),
                        stop=(kk == T // 2 - 1),
                        perf_mode=mybir.MatmulPerfMode.DoubleRow,
                    )
            else:
                for k in range(T):
                    nc.tensor.matmul(
                        dg[:],
                        ones8[:, 0, 0:1],
                        a8[:, k, :],
                        start=(k == 0),
                        stop=(k == T - 1),
                    )
            for m in range(T):
                ps = pps.tile([P, N], dt.float32)
                if fp8:
                    for kk in range(T // 2):
                        nc.tensor.matmul(
                            ps[:],
                            a8[:, 2 * kk : 2 * kk + 2, m * P : (m + 1) * P],
                            a8[:, 2 * kk : 2 * kk + 2, :],
                            start=(kk == 0),
                            stop=(kk == T // 2 - 1),
                            perf_mode=mybir.MatmulPerfMode.DoubleRow,
                        )
                else:
                    for k in range(T):
                        nc.tensor.matmul(
                            ps[:],
                            a8[:, k, m * P : (m + 1) * P],
                            a8[:, k, :],
                            start=(k == 0),
                            stop=(k == T - 1),
                        )
                x = px.tile([P, N], dt.bfloat16)
                # X = (A^2 * 1.0) * A ; accum_out = rowsum(X) = tri2
                nc.vector.scalar_tensor_tensor(
                    x[:],
                    ps[:],
                    1.0,
                    af[:, m, :],
                    op0=mybir.AluOpType.mult,
                    op1=mybir.AluOpType.mult,
                    accum_out=st_all[:, g * T + m : g * T + m + 1],
                )
            nc.scalar.copy(dg_all[:, g * N : (g + 1) * N], dg[:])
            if (g + 1) % 8 == 0 or g == gp - 1:
                g0 = (g // 8) * 8
                nc.sync.dma_start(
                    out=ot[:, g0 * T : (g + 1) * T],
                    in_=st_all[:, g0 * T : (g + 1) * T],
                )
                nc.sync.dma_start(
                    out=od[g0 * N : (g + 1) * N].rearrange(
                        "(o f) -> o f", o=1
                    ),
                    in_=dg_all[:, g0 * N : (g + 1) * N],
                )
    return nc


def _get_nc():
    key = (GP, MM_DTYPE)
    if key not in _NC_CACHE:
        _NC_CACHE[key] = build_nc(*key)
    return _NC_CACHE[key]


def _permute_shard(shard):
    # [gp, 512, 512] -> [gp, 128, 4, 512] with [g, p, t, n] = A[g, t*128+p, n]
    gp = shard.shape[0]
    return np.ascontiguousarray(
        shard.reshape(gp, T, P, N).transpose(0, 2, 1, 3), dtype=np.float32
    )


def run_device(adj_1, adj_2, trace=False):
    """Run the bass kernel on 8 cores; returns (tri2, deg) for each input
    tensor as [B, N] f32 arrays, plus the BassKernelResults."""
    from concourse.bass_utils import run_bass_kernel_spmd

    nc = _get_nc()
    in_maps = []
    for c in range(N_CORES):
        shard = np.concatenate(
            [adj_1[c * PER : (c + 1) * PER], adj_2[c * PER : (c + 1) * PER]],
            axis=0,
        )
        in_maps.append({"a": _permute_shard(shard)})
    res = run_bass_kernel_spmd(nc, in_maps, list(range(N_CORES)), trace=trace)
    # ot [128, gp*4]: node m*128+p of graph g at ot[p, g*4+m]
    tri = np.stack(
        [
            r["ot"].reshape(P, GP, T).transpose(1, 2, 0).reshape(GP, N)
            for r in res.results
        ]
    )
    deg = np.stack([r["od"].reshape(GP, N) for r in res.results])
    tri2_1 = tri[:, :PER].reshape(B, N)
    tri2_2 = tri[:, PER:].reshape(B, N)
    deg_1 = deg[:, :PER].reshape(B, N)
    deg_2 = deg[:, PER:].reshape(B, N)
    return (tri2_1, deg_1), (tri2_2, deg_2), res


def _hist(tri2, deg):
    # bit-exact f32 replication of the reference binning
    tri2 = tri2.astype(np.float32)
    deg = deg.astype(np.float32)
    denom = deg * (deg - np.float32(1.0))
    c = np.where(
        denom > 0,
        tri2 / np.maximum(denom, np.float32(1.0)),
        np.float32(0.0),
    ).astype(np.float32)
    idx = np.clip((c * np.float32(BINS)).astype(np.int32), 0, BINS - 1)
    hist = np.zeros((idx.shape[0], BINS), np.float32)
    np.add.at(hist, (np.arange(idx.shape[0])[:, None], idx), np.float32(1.0))
    return hist


def _mmd(x, y):
    x = x.astype(np.float64)
    y = y.astype(np.float64)

    def kmat(a, b):
        sq = (
            (a * a).sum(-1)[:, None]
            + (b * b).sum(-1)[None, :]
            - 2.0 * (a @ b.T)
        )
        return np.exp(-np.maximum(sq, 0.0) / (2.0 * SIGMA * SIGMA))

    return kmat(x, x).mean() + kmat(y, y).mean() - 2.0 * kmat(x, y).mean()


def kernel(adj_1, adj_2):
    (t1, d1), (t2, d2), _ = run_device(adj_1, adj_2)
    h1 = _hist(t1, d1)
    h2 = _hist(t2, d2)
    return np.float32(_mmd(h1, h2))


# revision 16
# speedup vs baseline: 1.0600x; 1.0600x over previous
"""Trainium2 Bass kernel for ClusteringMMD.

Per graph (batch-sharded 16+16 graphs onto each of 8 cores):
  - host pre-permutes the [512,512] adjacency to [128,4,512] so the
    device DMA is one fully contiguous 1MB transfer
  - ScalarE: one 2048-wide cast f32 -> fp8e4 (values are exactly 0/1)
  - TensorE: A^2 = A @ A via fp8 DoubleRow matmuls into PSUM (exact:
    0/1 products, fp32 accumulate); deg = ones^T @ A via two more
    DoubleRow matmuls (column-sum = row-sum for symmetric A)
  - VectorE: scalar_tensor_tensor fuses X = A^2 * A with accum_out =
    row-sum(X), yielding tri2 = diag(A^3) per node in one pass
  - DMA out tri2 [128,4] and deg [1,512] per graph
Host: bit-exact f32 replication of the reference's clustering-coefficient
binning (tri2/deg are exact small integers, so the device result is exact),
then the tiny [128,100] histogram MMD in f64.

The walrus build in this container rejects instructions carrying more than
one sync wait; _patch_compiler_wait_split() rewrites the BIR JSON right
before compilation, moving excess waits onto same-engine NoOps inserted
immediately before the over-subscribed instruction.
"""

import json
import numpy as np

B = 128
N = 512
BINS = 100
SIGMA = 1.0
N_CORES = 8
PER = B // N_CORES          # graphs per input tensor per core
GP = 2 * PER                # graphs per core (adj_1 shard + adj_2 shard)
P = 128
T = N // P                  # 4 row-blocks

MM_DTYPE = "fp8"            # "fp8" (DoubleRow) or "bf16"
WAIT_CAP = 1                # max sync waits this walrus accepts per inst

_NC_CACHE = {}


def _split_waits(bir_json, cap=WAIT_CAP):
    """Rewrite BIR JSON so no instruction carries more than `cap` sync
    waits; excess waits move to NoOps inserted just before it on the same
    engine (per-engine program order is list order within a block)."""
    m = json.loads(bir_json)
    ctr = 0
    for fn in m.get("functions", []):
        for blk in fn.get("blocks", []):
            out = []
            changed = False
            for ins in blk.get("instructions", []):
                si = ins.get("sync_info")
                waits = (si or {}).get("on_wait") or []
                if len(waits) > cap:
                    changed = True
                    for i in range(0, len(waits) - cap, cap):
                        ctr += 1
                        out.append(
                            {
                                "debug": ins.get("debug", 0),
                                "engine": ins["engine"],
                                "ins": [],
                                "name": f"WSPLIT-{ctr}",
                                "opcode": "NoOp",
                                "outs": [],
                                "text_hint": "wait_split",
                                "sync_info": {
                                    "on_wait": waits[i : i + cap],
                                    "on_update": [],
                                },
                            }
                        )
                    si["on_wait"] = waits[len(waits) - cap :]
                out.append(ins)
            if changed:
                blk["instructions"] = out
    return json.dumps(m).encode()


def _patch_compiler_wait_split():
    import concourse.bass_utils as bu
    import concourse.bass2jax as b2j

    if getattr(bu, "_wait_split_patched", False):
        return
    orig = bu.compile_bir_kernel

    def wrapped(bir_json, tmpdir, neff_name="file.neff"):
        return orig(_split_waits(bir_json), tmpdir, neff_name)

    bu.compile_bir_kernel = wrapped
    b2j.compile_bir_kernel = wrapped
    bu._wait_split_patched = True


def _patch_lean_tail():
    """Drop the second all-engine barrier of the Tile epilogue: semaphores
    are cleared after the first barrier (all engines quiesced), and nothing
    executes after the epilogue within this NEFF iteration."""
    from concourse.tile import TileContext
    from concourse.vector_clock import ScopedClock

    if getattr(TileContext, "_lean_tail", False):
        return

    def patched(self, tick_clock, wait_clock):
        nc = self.nc
        drain_inst = nc.sync.drain()
        wait_clock.add_sem_waits(
            drain_inst.ins, ScopedClock({None: tick_clock.global_clock})
        )
        nc.all_engine_barrier()
        assert self.sems is not None
        popped = nc._tile_sem_poison_stack.pop()
        assert popped is self._sem_poison
        nc.clear_and_free_semaphores(list(self.sems.allocated().values()))

    TileContext._drain_and_barrier = patched
    TileContext._lean_tail = True


def build_nc(gp=GP, mm_dtype=MM_DTYPE):
    import concourse.bass as bass
    import concourse.mybir as mybir
    from concourse.tile import TileContext
    from contextlib import ExitStack

    _patch_compiler_wait_split()
    _patch_lean_tail()
    dt = mybir.dt
    fp8 = mm_dtype == "fp8"
    cast_dt = dt.float8e4 if fp8 else dt.bfloat16

    nc = bass.Bass(
        "TRN2", target_bir_lowering=False, debug=False, num_devices=N_CORES
    )
    # input pre-permuted on host: a[g, p, t, n] = A_g[t*128 + p, n]
    a = nc.declare_dram_parameter("a", [gp, P, T, N], dt.float32, isOutput=False)
    # tri2 partition-major: ot[p, g*T + m] = tri2_g[m*128 + p]
    ot = nc.declare_dram_parameter("ot", [P, gp * T], dt.float32, isOutput=True)
    od = nc.declare_dram_parameter("od", [gp * N], dt.float32, isOutput=True)

    with TileContext(nc) as tc, ExitStack() as ctx:
        pconst = ctx.enter_context(tc.tile_pool(name="const", bufs=1))
        paf = ctx.enter_context(tc.tile_pool(name="af", bufs=8))
        pa8 = ctx.enter_context(tc.tile_pool(name="a8", bufs=4))
        px = ctx.enter_context(tc.tile_pool(name="xs", bufs=2))
        pps = ctx.enter_context(tc.tile_pool(name="ps", bufs=6, space="PSUM"))
        pdg = ctx.enter_context(tc.tile_pool(name="dg", bufs=2, space="PSUM"))

        # all-ones stationary operand for the deg column-sum matmuls;
        # [128, 2, 16] so the DoubleRow Ko-pair step is 16 bytes
        ones8 = pconst.tile([P, 2, 16], cast_dt)
        nc.vector.memset(ones8[:], 1.0)
        # whole-core result staging, one output DMA each at the end
        st_all = pconst.tile([P, gp * T], dt.float32)
        dg_all = pconst.tile([1, gp * N], dt.float32)

        for g in range(gp):
            af = paf.tile([P, T, N], dt.float32)
            nc.sync.dma_start(out=af[:], in_=a[g])
            a8 = pa8.tile([P, T, N], cast_dt)
            nc.scalar.activation(
                a8[:, :, :],
                af[:, :, :],
                mybir.ActivationFunctionType.Copy,
            )
            # deg = column-sum(A) (= row-sum, A symmetric) on the PE
            dg = pdg.tile([1, N], dt.float32)
            if fp8:
                for kk in range(T // 2):
                    nc.tensor.matmul(
                        dg[:],
                        ones8[:, :, 0:1],
                        a8[:, 2 * kk : 2 * kk + 2, :],
                        start=(kk == 0),
                        stop=(kk == T // 2 - 1),
                        perf_mode=mybir.MatmulPerfMode.DoubleRow,
                    )
            else:
                for k in range(T):
                    nc.tensor.matmul(
                        dg[:],
                        ones8[:, 0, 0:1],
                        a8[:, k, :],
                        start=(k == 0),
                        stop=(k == T - 1),
                    )
            for m in range(T):
                ps = pps.tile([P, N], dt.float32)
                if fp8:
                    for kk in range(T // 2):
                        nc.tensor.matmul(
                            ps[:],
                            a8[:, 2 * kk : 2 * kk + 2, m * P : (m + 1) * P],
                            a8[:, 2 * kk : 2 * kk + 2, :],
                            start=(kk == 0),
                            stop=(kk == T // 2 - 1),
                            perf_mode=mybir.MatmulPerfMode.DoubleRow,
                        )
                else:
                    for k in range(T):
                        nc.tensor.matmul(
                            ps[:],
                            a8[:, k, m * P : (m + 1) * P],
                            a8[:, k, :],
                            start=(k == 0),
                            stop=(k == T - 1),
                        )
                x = px.tile([P, N], dt.bfloat16)
                # X = (A^2 * 1.0) * A ; accum_out = rowsum(X) = tri2
                nc.vector.scalar_tensor_tensor(
                    x[:],
                    ps[:],
                    1.0,
                    af[:, m, :],
                    op0=mybir.AluOpType.mult,
                    op1=mybir.AluOpType.mult,
                    accum_out=st_all[:, g * T + m : g * T + m + 1],
                )
            nc.scalar.copy(dg_all[:, g * N : (g + 1) * N], dg[:])
            if (g + 1) % 8 == 0 or g == gp - 1:
                g0 = (g // 8) * 8
                # SWDGE so these don't head-of-line block the input queues
                nc.gpsimd.dma_start(
                    out=ot[:, g0 * T : (g + 1) * T],
                    in_=st_all[:, g0 * T : (g + 1) * T],
                )
                nc.gpsimd.dma_start(
                    out=od[g0 * N : (g + 1) * N].rearrange(
                        "(o f) -> o f", o=1
                    ),
                    in_=dg_all[:, g0 * N : (g + 1) * N],
                )
    return nc


def _get_nc():
    key = (GP, MM_DTYPE)
    if key not in _NC_CACHE:
        _NC_CACHE[key] = build_nc(*key)
    return _NC_CACHE[key]


def _permute_shard(shard):
    # [gp, 512, 512] -> [gp, 128, 4, 512] with [g, p, t, n] = A[g, t*128+p, n]
    gp = shard.shape[0]
    return np.ascontiguousarray(
        shard.reshape(gp, T, P, N).transpose(0, 2, 1, 3), dtype=np.float32
    )


def run_device(adj_1, adj_2, trace=False):
    """Run the bass kernel on 8 cores; returns (tri2, deg) for each input
    tensor as [B, N] f32 arrays, plus the BassKernelResults."""
    from concourse.bass_utils import run_bass_kernel_spmd

    nc = _get_nc()
    in_maps = []
    for c in range(N_CORES):
        shard = np.concatenate(
            [adj_1[c * PER : (c + 1) * PER], adj_2[c * PER : (c + 1) * PER]],
            axis=0,
        )
        in_maps.append({"a": _permute_shard(shard)})
    res = run_bass_kernel_spmd(nc, in_maps, list(range(N_CORES)), trace=trace)
    # ot [128, gp*4]: node m*128+p of graph g at ot[p, g*4+m]
    tri = np.stack(
        [
            r["ot"].reshape(P, GP, T).transpose(1, 2, 0).reshape(GP, N)
            for r in res.results
        ]
    )
    deg = np.stack([r["od"].reshape(GP, N) for r in res.results])
    tri2_1 = tri[:, :PER].reshape(B, N)
    tri2_2 = tri[:, PER:].reshape(B, N)
    deg_1 = deg[:, :PER].reshape(B, N)
    deg_2 = deg[:, PER:].reshape(B, N)
    return (tri2_1, deg_1), (tri2_2, deg_2), res


def _hist(tri2, deg):
    # bit-exact f32 replication of the reference binning
    tri2 = tri2.astype(np.float32)
    deg = deg.astype(np.float32)
    denom = deg * (deg - np.float32(1.0))
    c = np.where(
        denom > 0,
        tri2 / np.maximum(denom, np.float32(1.0)),
        np.float32(0.0),
    ).astype(np.float32)
    idx = np.clip((c * np.float32(BINS)).astype(np.int32), 0, BINS - 1)
    hist = np.zeros((idx.shape[0], BINS), np.float32)
    np.add.at(hist, (np.arange(idx.shape[0])[:, None], idx), np.float32(1.0))
    return hist


def _mmd(x, y):
    x = x.astype(np.float64)
    y = y.astype(np.float64)

    def kmat(a, b):
        sq = (
            (a * a).sum(-1)[:, None]
            + (b * b).sum(-1)[None, :]
            - 2.0 * (a @ b.T)
        )
        return np.exp(-np.maximum(sq, 0.0) / (2.0 * SIGMA * SIGMA))

    return kmat(x, x).mean() + kmat(y, y).mean() - 2.0 * kmat(x, y).mean()


def kernel(adj_1, adj_2):
    (t1, d1), (t2, d2), _ = run_device(adj_1, adj_2)
    h1 = _hist(t1, d1)
    h2 = _hist(t2, d2)
    return np.float32(_mmd(h1, h2))


# revision 21
# speedup vs baseline: 1.0689x; 1.0084x over previous
"""Trainium2 Bass kernel for ClusteringMMD.

Per graph (batch-sharded 16+16 graphs onto each of 8 cores):
  - host pre-permutes the [512,512] adjacency to [128,4,512] so the
    device DMA is one fully contiguous 1MB transfer
  - ScalarE: one 2048-wide cast f32 -> fp8e4 (values are exactly 0/1)
  - TensorE: A^2 = A @ A via fp8 DoubleRow matmuls into PSUM (exact:
    0/1 products, fp32 accumulate); deg = ones^T @ A via two more
    DoubleRow matmuls (column-sum = row-sum for symmetric A)
  - VectorE: scalar_tensor_tensor fuses X = A^2 * A with accum_out =
    row-sum(X), yielding tri2 = diag(A^3) per node in one pass
  - DMA out tri2 [128,4] and deg [1,512] per graph
Host: bit-exact f32 replication of the reference's clustering-coefficient
binning (tri2/deg are exact small integers, so the device result is exact),
then the tiny [128,100] histogram MMD in f64.

The walrus build in this container rejects instructions carrying more than
one sync wait; _patch_compiler_wait_split() rewrites the BIR JSON right
before compilation, moving excess waits onto same-engine NoOps inserted
immediately before the over-subscribed instruction.
"""

import json
import numpy as np

B = 128
N = 512
BINS = 100
SIGMA = 1.0
N_CORES = 8
PER = B // N_CORES          # graphs per input tensor per core
GP = 2 * PER                # graphs per core (adj_1 shard + adj_2 shard)
P = 128
T = N // P                  # 4 row-blocks

MM_DTYPE = "fp8"            # "fp8" (DoubleRow) or "bf16"
WAIT_CAP = 1                # max sync waits this walrus accepts per inst

_NC_CACHE = {}


def _split_waits(bir_json, cap=WAIT_CAP):
    """Rewrite BIR JSON so no instruction carries more than `cap` sync
    waits; excess waits move to NoOps inserted just before it on the same
    engine (per-engine program order is list order within a block)."""
    m = json.loads(bir_json)
    ctr = 0
    for fn in m.get("functions", []):
        for blk in fn.get("blocks", []):
            out = []
            changed = False
            for ins in blk.get("instructions", []):
                si = ins.get("sync_info")
                waits = (si or {}).get("on_wait") or []
                if len(waits) > cap:
                    changed = True
                    for i in range(0, len(waits) - cap, cap):
                        ctr += 1
                        out.append(
                            {
                                "debug": ins.get("debug", 0),
                                "engine": ins["engine"],
                                "ins": [],
                                "name": f"WSPLIT-{ctr}",
                                "opcode": "NoOp",
                                "outs": [],
                                "text_hint": "wait_split",
                                "sync_info": {
                                    "on_wait": waits[i : i + cap],
                                    "on_update": [],
                                },
                            }
                        )
                    si["on_wait"] = waits[len(waits) - cap :]
                out.append(ins)
            if changed:
                blk["instructions"] = out
    return json.dumps(m).encode()


def _patch_compiler_wait_split():
    import concourse.bass_utils as bu
    import concourse.bass2jax as b2j

    if getattr(bu, "_wait_split_patched", False):
        return
    orig = bu.compile_bir_kernel

    def wrapped(bir_json, tmpdir, neff_name="file.neff"):
        return orig(_split_waits(bir_json), tmpdir, neff_name)

    bu.compile_bir_kernel = wrapped
    b2j.compile_bir_kernel = wrapped
    bu._wait_split_patched = True





def build_nc(gp=GP, mm_dtype=MM_DTYPE):
    import concourse.bass as bass
    import concourse.mybir as mybir
    from concourse.tile import TileContext
    from contextlib import ExitStack

    _patch_compiler_wait_split()
    dt = mybir.dt
    fp8 = mm_dtype == "fp8"
    cast_dt = dt.float8e4 if fp8 else dt.bfloat16

    nc = bass.Bass(
        "TRN2", target_bir_lowering=False, debug=False, num_devices=N_CORES
    )
    # input pre-permuted on host: a[g, p, t, n] = A_g[t*128 + p, n]
    a = nc.declare_dram_parameter("a", [gp, P, T, N], dt.float32, isOutput=False)
    # tri2 partition-major: ot[p, g*T + m] = tri2_g[m*128 + p]
    ot = nc.declare_dram_parameter("ot", [P, gp * T], dt.float32, isOutput=True)
    od = nc.declare_dram_parameter("od", [gp * N], dt.float32, isOutput=True)

    with TileContext(nc) as tc, ExitStack() as ctx:
        pconst = ctx.enter_context(tc.tile_pool(name="const", bufs=1))
        paf = ctx.enter_context(tc.tile_pool(name="af", bufs=8))
        pa8 = ctx.enter_context(tc.tile_pool(name="a8", bufs=4))
        px = ctx.enter_context(tc.tile_pool(name="xs", bufs=2))
        pps = ctx.enter_context(tc.tile_pool(name="ps", bufs=5, space="PSUM"))
        pdg = ctx.enter_context(tc.tile_pool(name="dg", bufs=3, space="PSUM"))

        # all-ones stationary operand for the deg column-sum matmuls;
        # [128, 2, 16] so the DoubleRow Ko-pair step is 16 bytes
        ones8 = pconst.tile([P, 2, 16], cast_dt)
        nc.vector.memset(ones8[:], 1.0)
        # whole-core result staging, one output DMA each at the end
        st_all = pconst.tile([P, gp * T], dt.float32)
        dg_all = pconst.tile([1, gp * N], dt.float32)

        pending_dg = []  # deg PSUM tiles copied one graph late (no ACT stall)

        def flush_dg():
            while pending_dg:
                gq, dgq = pending_dg.pop(0)
                nc.scalar.copy(dg_all[:, gq * N : (gq + 1) * N], dgq[:])

        for g in range(gp):
            af = paf.tile([P, T, N], dt.float32)
            nc.sync.dma_start(out=af[:], in_=a[g])
            a8 = pa8.tile([P, T, N], cast_dt)
            nc.scalar.activation(
                a8[:, :, :],
                af[:, :, :],
                mybir.ActivationFunctionType.Copy,
            )
            flush_dg()
            # deg = column-sum(A) (= row-sum, A symmetric) on the PE
            dg = pdg.tile([1, N], dt.float32)
            if fp8:
                for kk in range(T // 2):
                    nc.tensor.matmul(
                        dg[:],
                        ones8[:, :, 0:1],
                        a8[:, 2 * kk : 2 * kk + 2, :],
                        start=(kk == 0),
                        stop=(kk == T // 2 - 1),
                        perf_mode=mybir.MatmulPerfMode.DoubleRow,
                    )
            else:
                for k in range(T):
                    nc.tensor.matmul(
                        dg[:],
                        ones8[:, 0, 0:1],
                        a8[:, k, :],
                        start=(k == 0),
                        stop=(k == T - 1),
                    )
            for m in range(T):
                ps = pps.tile([P, N], dt.float32)
                if fp8:
                    for kk in range(T // 2):
                        nc.tensor.matmul(
                            ps[:],
                            a8[:, 2 * kk : 2 * kk + 2, m * P : (m + 1) * P],
                            a8[:, 2 * kk : 2 * kk + 2, :],
                            start=(kk == 0),
                            stop=(kk == T // 2 - 1),
                            perf_mode=mybir.MatmulPerfMode.DoubleRow,
                        )
                else:
                    for k in range(T):
                        nc.tensor.matmul(
                            ps[:],
                            a8[:, k, m * P : (m + 1) * P],
                            a8[:, k, :],
                            start=(k == 0),
                            stop=(k == T - 1),
                        )
                x = px.tile([P, N], dt.bfloat16)
                # X = (A^2 * 1.0) * A ; accum_out = rowsum(X) = tri2
                nc.vector.scalar_tensor_tensor(
                    x[:],
                    ps[:],
                    1.0,
                    af[:, m, :],
                    op0=mybir.AluOpType.mult,
                    op1=mybir.AluOpType.mult,
                    accum_out=st_all[:, g * T + m : g * T + m + 1],
                )
            pending_dg.append((g, dg))
            if g == gp - 1:
                flush_dg()
            if (g + 1) % 8 == 0 or g == gp - 1:
                g0 = (g // 8) * 8
                # SWDGE so these don't head-of-line block the input queues
                nc.gpsimd.dma_start(
                    out=ot[:, g0 * T : (g + 1) * T],
                    in_=st_all[:, g0 * T : (g + 1) * T],
                )
                nc.gpsimd.dma_start(
                    out=od[g0 * N : (g + 1) * N].rearrange(
                        "(o f) -> o f", o=1
                    ),
                    in_=dg_all[:, g0 * N : (g + 1) * N],
                )
    return nc


def _get_nc():
    key = (GP, MM_DTYPE)
    if key not in _NC_CACHE:
        _NC_CACHE[key] = build_nc(*key)
    return _NC_CACHE[key]


def _permute_shard(shard):
    # [gp, 512, 512] -> [gp, 128, 4, 512] with [g, p, t, n] = A[g, t*128+p, n]
    gp = shard.shape[0]
    return np.ascontiguousarray(
        shard.reshape(gp, T, P, N).transpose(0, 2, 1, 3), dtype=np.float32
    )


def run_device(adj_1, adj_2, trace=False):
    """Run the bass kernel on 8 cores; returns (tri2, deg) for each input
    tensor as [B, N] f32 arrays, plus the BassKernelResults."""
    from concourse.bass_utils import run_bass_kernel_spmd

    nc = _get_nc()
    in_maps = []
    for c in range(N_CORES):
        shard = np.concatenate(
            [adj_1[c * PER : (c + 1) * PER], adj_2[c * PER : (c + 1) * PER]],
            axis=0,
        )
        in_maps.append({"a": _permute_shard(shard)})
    res = run_bass_kernel_spmd(nc, in_maps, list(range(N_CORES)), trace=trace)
    # ot [128, gp*4]: node m*128+p of graph g at ot[p, g*4+m]
    tri = np.stack(
        [
            r["ot"].reshape(P, GP, T).transpose(1, 2, 0).reshape(GP, N)
            for r in res.results
        ]
    )
    deg = np.stack([r["od"].reshape(GP, N) for r in res.results])
    tri2_1 = tri[:, :PER].reshape(B, N)
    tri2_2 = tri[:, PER:].reshape(B, N)
    deg_1 = deg[:, :PER].reshape(B, N)
    deg_2 = deg[:, PER:].reshape(B, N)
    return (tri2_1, deg_1), (tri2_2, deg_2), res


def _hist(tri2, deg):
    # bit-exact f32 replication of the reference binning
    tri2 = tri2.astype(np.float32)
    deg = deg.astype(np.float32)
    denom = deg * (deg - np.float32(1.0))
    c = np.where(
        denom > 0,
        tri2 / np.maximum(denom, np.float32(1.0)),
        np.float32(0.0),
    ).astype(np.float32)
    idx = np.clip((c * np.float32(BINS)).astype(np.int32), 0, BINS - 1)
    hist = np.zeros((idx.shape[0], BINS), np.float32)
    np.add.at(hist, (np.arange(idx.shape[0])[:, None], idx), np.float32(1.0))
    return hist


def _mmd(x, y):
    x = x.astype(np.float64)
    y = y.astype(np.float64)

    def kmat(a, b):
        sq = (
            (a * a).sum(-1)[:, None]
            + (b * b).sum(-1)[None, :]
            - 2.0 * (a @ b.T)
        )
        return np.exp(-np.maximum(sq, 0.0) / (2.0 * SIGMA * SIGMA))

    return kmat(x, x).mean() + kmat(y, y).mean() - 2.0 * kmat(x, y).mean()


def kernel(adj_1, adj_2):
    (t1, d1), (t2, d2), _ = run_device(adj_1, adj_2)
    h1 = _hist(t1, d1)
    h2 = _hist(t2, d2)
    return np.float32(_mmd(h1, h2))
